# revision 9
# baseline (speedup 1.0000x reference)
"""Trainium2 Bass kernel for FeatureEmbedding (GNN message passing).

Pipeline (8 NeuronCores):
  - batch n=16 sharded 2/core for embed-gather, softmax-attention, output
  - GCN edges sharded by target-node range (128 nodes/core) so the
    scatter reduction is local: one-hot selection matmuls in PSUM
  - two small collectives: AllGather of {hW slice, dis}, AllToAll of
    pre-softmax attention rows
"""

import os

import numpy as np

T = 1024
NB = 16
D = 512
F = 8
VOC = 50000
NCORE = 8
NL = NB // NCORE  # 2 batch items per core
TJ = T // 128  # 8 t-tiles
CH = 1024  # max idxs per custom dma_gather (SWDGE ring limit)

_cache = {}


def _build_module(nep):
    import concourse.bacc as bacc
    import concourse.bass as bass
    import concourse.mybir as mybir
    from concourse.masks import make_identity
    from concourse.tile import TileContext

    f32 = mybir.dt.float32
    nch = nep // 128  # number of 128-edge chunks

    nc = bacc.Bacc("TRN2", target_bir_lowering=False, debug=False,
                   num_devices=NCORE)

    emb = nc.dram_tensor("emb", [VOC, D], f32, kind="ExternalInput")
    xid = nc.dram_tensor("xid", [128, NL * TJ], mybir.dt.int32,
                         kind="ExternalInput")
    pe_in = nc.dram_tensor("pe_in", [T, D], f32, kind="ExternalInput")
    w_in = nc.dram_tensor("w_in", [D, F], f32, kind="ExternalInput")
    bt_in = nc.dram_tensor("bt_in", [128, 128], f32, kind="ExternalInput")
    io_in = nc.dram_tensor("io_in", [128, 128], f32, kind="ExternalInput")
    ct_in = nc.dram_tensor("ct_in", [1, D], f32, kind="ExternalInput")
    gi_in = nc.dram_tensor("gi_in", [128, nep // 16], mybir.dt.int16,
                           kind="ExternalInput")
    cv_in = nc.dram_tensor("cv_in", [128, nch], f32, kind="ExternalInput")

    att_o = nc.dram_tensor("att_o", [NL * F, T], f32, kind="ExternalOutput")
    out_o = nc.dram_tensor("out_o", [NL * 9, D], f32, kind="ExternalOutput")

    g_d = nc.dram_tensor("g_d", [T + 1, 128], f32, kind="Internal")
    c1i = nc.dram_tensor("c1i", [T * NB + 128], f32, kind="Internal")
    c1o = nc.dram_tensor("c1o", [NCORE, T * NB + 128], f32, kind="Internal",
                         addr_space="Shared")
    # c2i/c2o: 8 shards of [128 nodes x 16 (n,f) cols]
    c2i = nc.dram_tensor("c2i", [NCORE, 128 * NL * F], f32, kind="Internal")
    c2o = nc.dram_tensor("c2o", [NCORE, 128 * NL * F], f32, kind="Internal")

    rg = [list(range(NCORE))]

    with TileContext(nc) as tc:
        with tc.tile_pool(name="cpool", bufs=1) as cp, \
             tc.tile_pool(name="wpool", bufs=4) as wp:
            # ---- constant / persistent tiles ----
            xid_t = cp.tile([128, NL * TJ], mybir.dt.int32)
            w_t = cp.tile([128, 4 * F], f32)
            bt_t = cp.tile([128, 128], f32)
            io_t = cp.tile([128, 128], f32)
            ct_t = cp.tile([1, D], f32)
            gi_t = cp.tile([128, nep // 16], mybir.dt.int16)
            cv_t = cp.tile([128, nch], f32)
            pe_t = cp.tile([128, TJ * D], f32)
            enc_t = cp.tile([128, NL * TJ * D], f32)
            sel_t = cp.tile([128, nep], f32)
            msg_t = cp.tile([128, nep], f32)
            g_sb = cp.tile([128, TJ * 128], f32)
            hw_sb = cp.tile([128, TJ * NB], f32)
            ea_t = cp.tile([128, TJ * NL * F], f32)
            eaE_t = cp.tile([128, TJ * NL * F], f32)
            att_sb = [cp.tile([F, T], f32, tag=f"att{i}", name=f"att_sb{i}") for i in range(NL)]
            ident = cp.tile([128, 128], f32)
            ones_t = cp.tile([128, 1], f32)
            dis_l = cp.tile([128, 1], f32)
            dis8 = cp.tile([128, TJ], f32)
            sums_sb = [cp.tile([F, 1], f32, tag=f"ss{i}", name=f"sums_sb{i}") for i in range(NL)]
            rec_t = [cp.tile([F, 1], f32, tag=f"rc{i}", name=f"rec_t{i}") for i in range(NL)]
            attpre = cp.tile([128, 128], f32)
            z1 = cp.tile([1, 128], f32)

            nc.sync.dma_start(out=xid_t[:], in_=xid[:])
            nc.sync.dma_start(
                out=w_t[:].rearrange("p (kt f) -> p kt f", f=F),
                in_=w_in[:].rearrange("(kt p) f -> p kt f", p=128))
            nc.sync.dma_start(out=bt_t[:], in_=bt_in[:])
            nc.sync.dma_start(out=io_t[:], in_=io_in[:])
            nc.sync.dma_start(out=ct_t[:], in_=ct_in[:])
            nc.sync.dma_start(out=gi_t[:], in_=gi_in[:])
            nc.sync.dma_start(out=cv_t[:], in_=cv_in[:])
            nc.sync.dma_start(
                out=pe_t[:].rearrange("p (j d) -> p j d", d=D),
                in_=pe_in[:].rearrange("(j p) d -> p j d", p=128))
            make_identity(nc, ident[:])
            nc.vector.memset(ones_t[:], 1.0)
            nc.vector.memset(z1[:], 0.0)

            # ---- phase A: selection matrices + degree ----
            for c in range(nch):
                nc.vector.tensor_tensor(
                    out=sel_t[:, c * 128:(c + 1) * 128],
                    in0=cv_t[:, c:c + 1].to_broadcast([128, 128]),
                    in1=io_t[:],
                    op=mybir.AluOpType.is_equal)

            with tc.tile_pool(name="psA", bufs=1, space="PSUM") as pa, \
                 tc.tile_pool(name="psT", bufs=2, space="PSUM") as pt:
                deg_ps = pa.tile([128, 1], f32)
                for c in range(nch):
                    nc.tensor.matmul(
                        out=deg_ps[:],
                        lhsT=sel_t[:, c * 128:(c + 1) * 128],
                        rhs=ones_t[:],
                        start=(c == 0), stop=(c == nch - 1))
                # dis = sqrt(1/deg)
                rdeg = wp.tile([128, 1], f32, tag="rdeg")
                nc.vector.reciprocal(out=rdeg[:], in_=deg_ps[:])
                nc.scalar.activation(out=dis_l[:], in_=rdeg[:],
                                     func=mybir.ActivationFunctionType.Sqrt)

                # ---- phase A: embedding gather + hW ----
                for jj in range(NL * TJ):
                    nc.gpsimd.indirect_dma_start(
                        out=enc_t[:, jj * D:(jj + 1) * D],
                        out_offset=None,
                        in_=emb[:],
                        in_offset=bass.IndirectOffsetOnAxis(
                            ap=xid_t[:, jj:jj + 1], axis=0))

                for jj in range(NL * TJ):
                    n_l, j = jj // TJ, jj % TJ
                    hw_ps = pt.tile([128, F], f32, tag="hw")
                    for c4 in range(4):
                        tp = pt.tile([128, 128], f32, tag="tp")
                        nc.tensor.transpose(
                            out=tp[:],
                            in_=enc_t[:, jj * D + c4 * 128:
                                      jj * D + (c4 + 1) * 128],
                            identity=ident[:])
                        encT = wp.tile([128, 128], f32, tag="encT")
                        nc.vector.tensor_copy(out=encT[:], in_=tp[:])
                        nc.tensor.matmul(
                            out=hw_ps[:],
                            lhsT=encT[:],
                            rhs=w_t[:, c4 * F:(c4 + 1) * F],
                            start=(c4 == 0), stop=(c4 == 3))
                    nc.vector.tensor_copy(
                        out=hw_sb[:, j * NB + n_l * F:j * NB + (n_l + 1) * F],
                        in_=hw_ps[:])

            nc.sync.dma_start(
                out=c1i[:T * NB].rearrange("(j p c) -> p j c", p=128, c=NB),
                in_=hw_sb[:].rearrange("p (j c) -> p j c", c=NB))
            nc.sync.dma_start(
                out=c1i[T * NB:].rearrange("(p one) -> p one", one=1),
                in_=dis_l[:])

            nc.gpsimd.collective_compute(
                "AllGather", mybir.AluOpType.bypass, replica_groups=rg,
                ins=[c1i[:]], outs=[c1o[:]])

            # ---- phase B: build g table, edge gather, scatter matmuls ----
            g_sb3 = g_sb[:].rearrange("p (j c) -> p j c", c=128)
            for r in range(NCORE):
                nc.sync.dma_start(
                    out=g_sb3[:, :, r * NB:(r + 1) * NB],
                    in_=c1o[r, :T * NB].rearrange("(j p c) -> p j c",
                                                  p=128, c=NB))
                nc.sync.dma_start(
                    out=dis8[:, r:r + 1],
                    in_=c1o[r, T * NB:].rearrange("(p one) -> p one", one=1))
            for j in range(TJ):
                nc.vector.tensor_scalar_mul(
                    g_sb[:, j * 128:(j + 1) * 128],
                    g_sb[:, j * 128:(j + 1) * 128],
                    dis8[:, j:j + 1])
            nc.sync.dma_start(
                out=g_d[0:T].rearrange("(j p) c -> p j c", p=128),
                in_=g_sb3)
            nc.sync.dma_start(out=g_d[T:T + 1, :], in_=z1[:])

            msg3 = msg_t[:].rearrange("p (s e) -> p s e", e=128)
            for c0 in range(0, nep, CH):
                nn = min(CH, nep - c0)
                nc.gpsimd.dma_gather(
                    out_ap=msg3[:, c0 // 128:(c0 + nn) // 128, :],
                    in_ap=g_d[:],
                    idxs_ap=gi_t[:, c0 // 16:(c0 + nn) // 16],
                    num_idxs=nn, num_idxs_reg=nn, elem_size=128)

            with tc.tile_pool(name="psB", bufs=1, space="PSUM") as pb:
                s_ps = pb.tile([128, 128], f32)
                for c in range(nch):
                    nc.tensor.matmul(
                        out=s_ps[:],
                        lhsT=sel_t[:, c * 128:(c + 1) * 128],
                        rhs=msg_t[:, c * 128:(c + 1) * 128],
                        start=(c == 0), stop=(c == nch - 1))
                nc.vector.tensor_scalar_mul(attpre[:], s_ps[:], dis_l[:, :1])
            nc.vector.tensor_add(out=attpre[:], in0=attpre[:], in1=bt_t[:])
            nc.sync.dma_start(
                out=c2i[:].rearrange("j (p c) -> p j c", p=128),
                in_=attpre[:].rearrange("p (j c) -> p j c", c=NB))

            nc.gpsimd.collective_compute(
                "AllToAll", mybir.AluOpType.bypass, replica_groups=rg,
                ins=[c2i[:]], outs=[c2o[:]])

            # ---- phase C: softmax-free attention + outputs ----
            nc.sync.dma_start(
                out=ea_t[:].rearrange("p (r c) -> p r c", c=NL * F),
                in_=c2o[:].rearrange("r (p c) -> p r c", p=128))
            nc.scalar.activation(out=eaE_t[:], in_=ea_t[:],
                                 func=mybir.ActivationFunctionType.Exp)

            with tc.tile_pool(name="psC", bufs=2, space="PSUM") as pc, \
                 tc.tile_pool(name="psC2", bufs=2, space="PSUM") as pc2:
                for n_l in range(NL):
                    sums_ps = pc.tile([F, 1], f32, tag="sums")
                    for j in range(TJ):
                        nc.tensor.matmul(
                            out=sums_ps[:],
                            lhsT=eaE_t[:, j * NL * F + n_l * F:
                                       j * NL * F + (n_l + 1) * F],
                            rhs=ones_t[:],
                            start=(j == 0), stop=(j == TJ - 1))
                    nc.vector.tensor_copy(out=sums_sb[n_l][:], in_=sums_ps[:])
                    nc.vector.reciprocal(out=rec_t[n_l][:], in_=sums_sb[n_l][:])

                for n_l in range(NL):
                    lr_ps = pc2.tile([F, D], f32, tag="lr")
                    for j in range(TJ):
                        nc.tensor.matmul(
                            out=lr_ps[:],
                            lhsT=eaE_t[:, j * NL * F + n_l * F:
                                       j * NL * F + (n_l + 1) * F],
                            rhs=enc_t[:, (n_l * TJ + j) * D:
                                      (n_l * TJ + j + 1) * D],
                            start=(j == 0), stop=False)
                    for j in range(TJ):
                        nc.tensor.matmul(
                            out=lr_ps[:],
                            lhsT=eaE_t[:, j * NL * F + n_l * F:
                                       j * NL * F + (n_l + 1) * F],
                            rhs=pe_t[:, j * D:(j + 1) * D],
                            start=False, stop=(j == TJ - 1))
                    lr_sb = wp.tile([F, D], f32, tag="lrsb")
                    nc.vector.tensor_scalar_mul(
                        lr_sb[:], lr_ps[:], rec_t[n_l][:, :1])
                    nc.sync.dma_start(
                        out=out_o[n_l * 9 + 1:(n_l + 1) * 9, :], in_=lr_sb[:])
                    nc.sync.dma_start(
                        out=out_o[n_l * 9:n_l * 9 + 1, :], in_=ct_t[:])

                for n_l in range(NL):
                    for j in range(TJ):
                        atp = pc2.tile([F, 128], f32, tag="atp")
                        nc.tensor.transpose(
                            out=atp[:],
                            in_=eaE_t[:, j * NL * F + n_l * F:
                                      j * NL * F + (n_l + 1) * F],
                            identity=ident[:])
                        nc.vector.tensor_scalar_mul(
                            att_sb[n_l][:, j * 128:(j + 1) * 128], atp[:],
                            rec_t[n_l][:, :1])
            for n_l in range(NL):
                nc.sync.dma_start(out=att_o[n_l * F:(n_l + 1) * F, :],
                                  in_=att_sb[n_l][:])

    nc.compile()
    return nc


def _marshal(inputs):
    x = np.asarray(inputs["x"]).astype(np.int64)            # [T, NB]
    ei = np.asarray(inputs["edge_index"]).astype(np.int64)  # [2, E]
    pe = np.asarray(inputs["PE"], dtype=np.float32).reshape(T, D)
    emb = np.ascontiguousarray(np.asarray(inputs["embed_table"],
                                          dtype=np.float32))
    ct = np.asarray(inputs["code_token"], dtype=np.float32).reshape(1, D)
    w = np.ascontiguousarray(np.asarray(inputs["W_gcn"], dtype=np.float32))
    b = np.asarray(inputs["b_gcn"], dtype=np.float32)

    rows, cols = ei[0], ei[1]
    per_core = []
    for k in range(NCORE):
        m = (cols // 128) == k
        r = np.concatenate([rows[m], 128 * k + np.arange(128)])
        c = np.concatenate([cols[m] - 128 * k, np.arange(128)])
        per_core.append((r, c))
    nep = max(len(r) for r, _ in per_core)
    nep = ((nep + 127) // 128) * 128

    bt = np.tile(b, (128, NB)).astype(np.float32)           # [128, 128]
    io = np.tile(np.arange(128, dtype=np.float32), (128, 1))

    in_maps = []
    for k in range(NCORE):
        r, c = per_core[k]
        pad = nep - len(r)
        r = np.concatenate([r, np.full(pad, T)]).astype(np.int16)
        c = np.concatenate([c, np.full(pad, -1)]).astype(np.float32)
        gi = np.tile(r.reshape(nep // 16, 16).T, (8, 1)).copy()
        cv = np.ascontiguousarray(c.reshape(nep // 128, 128).T)
        xidk = np.empty((128, NL * TJ), np.int32)
        for jj in range(NL * TJ):
            n_l, j = jj // TJ, jj % TJ
            xidk[:, jj] = x[j * 128:(j + 1) * 128, NL * k + n_l]
        in_maps.append({
            "emb": emb, "xid": xidk, "pe_in": pe, "w_in": w, "bt_in": bt,
            "io_in": io, "ct_in": ct, "gi_in": gi, "cv_in": cv,
        })
    return nep, in_maps


def kernel(**inputs):
    nep, in_maps = _marshal(inputs)
    if nep not in _cache:
        _cache[nep] = _build_module(nep)
    nc = _cache[nep]

    from concourse import bass_utils
    trace = bool(int(os.environ.get("KERNEL_TRACE", "0")))
    res = bass_utils.run_bass_kernel_spmd(
        nc, in_maps, core_ids=list(range(NCORE)), trace=trace)
    if trace and res.exec_time_ns is not None:
        print(f"HW exec time: {res.exec_time_ns} ns")
        kernel.last_exec_time_ns = res.exec_time_ns

    out = np.empty((NB, 9, D), np.float32)
    att = np.empty((NB, F, T), np.float32)
    for k in range(NCORE):
        out[NL * k:NL * (k + 1)] = res.results[k]["out_o"].reshape(NL, 9, D)
        att[NL * k:NL * (k + 1)] = res.results[k]["att_o"].reshape(NL, F, T)
    return out, att
